# revision 1
# baseline (speedup 1.0000x reference)
"""Trainium2 Bass kernel for PointCloudTeacher (2x EdgeConv with KNN graph).

Sharding: 8 NeuronCores, B=4 point clouds of N=4096 points; core c handles
(batch b = c//2, row-half h = c%2) = 2048 query rows. Per-core inputs are
column/row-PERMUTED so the core's own half comes first -- one SPMD program
serves all cores. Two launches with a host gather of x1 in between.

Per block:
  - Coarse KNN keys via single-pass TF32 (float32r) gram matmul plus a K=1
    rank-1 pass adding -|x_m|^2/2; top-8 via DVE max8/find_index8.
  - Exact re-rank of the 8 candidates: gather candidate rows (with a -sq/2
    column) and dot against the center row [x_n, 1] -> exact f32 keys.
    Slot 0 is the self row (copied from the center, not gathered).
  - y_a = (s*W_a) @ x for the full cloud; y_c = (s*W_c) @ x + b' for own
    rows (BN folded into the weights; max over k commutes with the monotone
    BN+LeakyReLU). y-matmul tiles are interleaved with the KNN tiles to
    keep the PE dense and warm.
  - Epilogue: self row of y_a direct-loaded + 4 indirect gathers by the
    exact top-4 indices, max over k via strided tensor_reduce, add y_c,
    LeakyReLU via Prelu(alpha=0.2).
Precision: block1 y-matmuls use a 3-pass tf32+bf16 hi/lo split (x1 feeds
KNN2 so it needs ~f32 quality); block2 y-matmuls are bf16 with bf16 y_a
storage (output-only path).
"""

import numpy as np
import ml_dtypes

import concourse.bass as bass
import concourse.bacc as bacc
import concourse.mybir as mybir
from concourse.tile import TileContext
from concourse.bass_utils import run_bass_kernel_spmd

dt = mybir.dt
AF = mybir.ActivationFunctionType
OP = mybir.AluOpType

P = 128
N = 4096
HALF = 2048
B = 4
C1, O1 = 512, 864
C2, O2 = 864, 1728
K = 5
NCAND = 8
EPS = 1e-5
SLOPE = 0.2
XA1 = 516   # rerank row width block1: x row + -sq/2 + pad
XA2 = 868   # rerank row width block2: x1 row + -sq/2 + pad
N_TILES = N // P        # 32
H_TILES = HALF // P     # 16


# ---------------------------------------------------------------- host utils

def _tf32(a):
    a = np.ascontiguousarray(a, dtype=np.float32)
    u = a.view(np.uint32).astype(np.uint64)
    u = ((u + 0x1000 + ((u >> 13) & 1)) & 0xFFFFE000).astype(np.uint32)
    return u.view(np.float32)


def _bf16(a):
    return np.ascontiguousarray(a, dtype=np.float32).astype(ml_dtypes.bfloat16)


def _stripes(xT, n_k):
    """(cin, N) -> (N_TILES, 128, n_k*128) tile-major lhsT stripes.
    [t, c, ci*128+col] = xT[ci*128+c, t*128+col]; rows past cin are zero."""
    cin, n = xT.shape
    out = np.zeros((N_TILES, P, n_k * P), xT.dtype)
    for ci in range(n_k):
        kk = min(P, cin - ci * P)
        blk = xT[ci * P:ci * P + kk, :].reshape(kk, N_TILES, P)
        out[:, :kk, ci * P:(ci + 1) * P] = blk.transpose(1, 0, 2)
    return out


def _mid_bcast(ap, rep):
    """(P, F) access pattern -> (P, rep, F) with 0-stride middle dim."""
    pat = [list(ap.ap[0]), [0, rep], list(ap.ap[1])]
    return bass.AP(ap.tensor, ap.offset, pat)


def _last_bcast(ap, rep):
    """(P, F) access pattern -> (P, F, rep) with 0-stride last dim."""
    pat = [list(ap.ap[0]), list(ap.ap[1]), [0, rep]]
    return bass.AP(ap.tensor, ap.offset, pat)


def _chunks(c):
    out = []
    o = 0
    while o < c:
        kk = min(P, c - o)
        out.append((o, kk))
        o += kk
    return out


# ---------------------------------------------------------------- program

def _build_block(cin, cout, xa_w, split_y):
    """split_y=True: block1 (3-pass tf32+bf16 y-matmuls, f32 y_a table).
    split_y=False: block2 (bf16 y-matmuls, bf16 y_a table)."""
    nc = bacc.Bacc("TRN2", target_bir_lowering=False)

    ksizes = _chunks(cin)
    n_k = len(ksizes)
    ytile = dt.bfloat16 if not split_y else dt.float32

    # ---- inputs
    xTr = nc.dram_tensor("xTr", [cin + 2, N], dt.float32r, kind="ExternalInput")
    xa = nc.dram_tensor("xa", [N, xa_w], dt.float32, kind="ExternalInput")
    io8 = nc.dram_tensor("io8", [P, NCAND], dt.float32, kind="ExternalInput")
    ones1 = nc.dram_tensor("ones1", [1, P], dt.float32r, kind="ExternalInput")
    if split_y:
        # bf16 lo/hi stripe streams for the cross passes
        xtl_s = nc.dram_tensor("xtl_s", [N_TILES, P, n_k * P], dt.bfloat16,
                               kind="ExternalInput")
        xthb_s = nc.dram_tensor("xthb_s", [N_TILES, P, n_k * P], dt.bfloat16,
                                kind="ExternalInput")
        wah = nc.dram_tensor("wah", [cin, cout], dt.float32r, kind="ExternalInput")
        wal = nc.dram_tensor("wal", [cin, cout], dt.bfloat16, kind="ExternalInput")
        wahb = nc.dram_tensor("wahb", [cin, cout], dt.bfloat16, kind="ExternalInput")
        wch = nc.dram_tensor("wch", [cin, cout], dt.float32r, kind="ExternalInput")
        wcl = nc.dram_tensor("wcl", [cin, cout], dt.bfloat16, kind="ExternalInput")
        wchb = nc.dram_tensor("wchb", [cin, cout], dt.bfloat16, kind="ExternalInput")
        b1b = nc.dram_tensor("b1b", [P, cout], dt.float32, kind="ExternalInput")
    else:
        # bf16 x1^T stripe stream for the y lhsT side
        x1b_s = nc.dram_tensor("x1b_s", [N_TILES, P, n_k * P], dt.bfloat16,
                               kind="ExternalInput")
        wah = nc.dram_tensor("wah", [cin, cout], dt.bfloat16, kind="ExternalInput")
        wch = nc.dram_tensor("wch", [cin + 1, cout], dt.bfloat16,
                             kind="ExternalInput")
        onesb = nc.dram_tensor("onesb", [1, P], dt.bfloat16, kind="ExternalInput")

    # internal dram + output
    yad = nc.dram_tensor("yad", [N, cout], ytile)
    ycd = nc.dram_tensor("ycd", [HALF, cout], dt.float32)
    xout = nc.dram_tensor("xout", [HALF, cout], dt.float32, kind="ExternalOutput")

    osub = [(o, min(512, cout - o)) for o in range(0, cout, 512)]
    if cout == O2:
        # subtiles aligned so [0:864] and [864:1728] split into two psum tiles
        osub = [(0, 512), (512, 352), (864, 512), (1376, 352)]

    with TileContext(nc) as tc:
        with tc.tile_pool(name="persist", bufs=1) as pp:
            xtr_t = []
            for ci, (off, kk) in enumerate(ksizes):
                t = pp.tile([kk, N], dt.float32r, tag=f"xtr{ci}")
                nc.sync.dma_start(t[:], xTr[off:off + kk, :])
                xtr_t.append(t)
            biasrow = pp.tile([1, N], dt.float32r, tag="biasrow")
            nc.sync.dma_start(biasrow[:], xTr[cin + 1:cin + 2, :])
            onest = pp.tile([1, P], dt.float32r, tag="onest")
            nc.sync.dma_start(onest[:], ones1[:])
            io8t = pp.tile([P, NCAND], dt.float32, tag="io8")
            nc.sync.dma_start(io8t[:], io8[:])
            if not split_y:
                onesbt = pp.tile([1, P], dt.bfloat16, tag="onesbt")
                nc.sync.dma_start(onesbt[:], onesb[:])

            # persistent exact-top4 index tiles, one per row tile
            pidx_cm = tc.tile_pool(name="pidx", bufs=H_TILES)
            pidx = pidx_cm.__enter__()
            idx4_t = []

            # ------------- combined phase: KNN(t) interleaved with y tiles
            with (
                tc.tile_pool(name="pw", bufs=1) as pw,
                tc.tile_pool(name="pstr", bufs=2) as pstr,
                tc.tile_pool(name="pac", bufs=2) as pac,
                tc.tile_pool(name="pacy", bufs=1) as pacy,
                tc.tile_pool(name="pk", bufs=2 if split_y else 1) as pk,
                tc.tile_pool(name="pbs", bufs=2) as pbs,
                tc.tile_pool(name="pbc", bufs=1) as pbc,
                tc.tile_pool(name="pgq", bufs=2, space="PSUM") as pgq,
                tc.tile_pool(name="pyp", bufs=2, space="PSUM") as pyp,
            ):
                # weight tiles
                def load_w(grp):
                    wsrc = wah if grp == "a" else wch
                    wdt = dt.float32r if split_y else dt.bfloat16
                    wh_t, wl_t, whb_t = [], [], []
                    for ci, (off, kk) in enumerate(ksizes):
                        wtag = f"w{grp}" if split_y else "w"
                        t = pw.tile([kk, cout], wdt, tag=f"{wtag}h{ci}")
                        nc.sync.dma_start(t[:], wsrc[off:off + kk, :])
                        wh_t.append(t)
                        if split_y:
                            wsl = wal if grp == "a" else wcl
                            wsb = wahb if grp == "a" else wchb
                            t = pw.tile([kk, cout], dt.bfloat16, tag=f"{wtag}l{ci}")
                            nc.sync.dma_start(t[:], wsl[off:off + kk, :])
                            wl_t.append(t)
                            t = pw.tile([kk, cout], dt.bfloat16, tag=f"{wtag}hb{ci}")
                            nc.sync.dma_start(t[:], wsb[off:off + kk, :])
                            whb_t.append(t)
                    return wh_t, wl_t, whb_t

                wa_trio = load_w("a")
                if split_y:
                    wc_trio = load_w("c")
                    b1t = pw.tile([P, cout], dt.float32, tag="b1t")
                    nc.sync.dma_start(b1t[:], b1b[:])
                else:
                    wcbias = pw.tile([1, cout], dt.bfloat16, tag="wcbias")
                    nc.sync.dma_start(wcbias[:], wch[cin:cin + 1, :])
                    wc_trio = None

                def y_tile(tile, grp, trio):
                    """one (128, cout) y tile: tile index into the cloud"""
                    cs = slice(tile * P, (tile + 1) * P)
                    wh_t, wl_t, whb_t = trio
                    if split_y:
                        stl = pstr.tile([P, n_k * P], dt.bfloat16, tag="stl")
                        nc.sync.dma_start(stl[:], xtl_s[tile])
                        sthb = pstr.tile([P, n_k * P], dt.bfloat16, tag="sthb")
                        nc.sync.dma_start(sthb[:], xthb_s[tile])
                    else:
                        stb = pstr.tile([P, n_k * P], dt.bfloat16, tag="stb")
                        nc.sync.dma_start(stb[:], x1b_s[tile])
                    # psum halves (<=864 wide each)
                    pshs = []
                    half_w = min(cout, 864)
                    for hh in range((cout + 863) // 864):
                        pshs.append(pyp.tile([P, half_w], dt.float32, tag="yps", name=f"yps{hh}"))
                    for (oo, ow) in osub:
                        hh, po = (0, oo) if oo < 864 else (1, oo - 864)
                        ps = pshs[hh]
                        if split_y:
                            for ci, (off, kk) in enumerate(ksizes):
                                nc.tensor.matmul(
                                    ps[:, po:po + ow],
                                    lhsT=xtr_t[ci][:kk, cs],
                                    rhs=wh_t[ci][:, oo:oo + ow],
                                    start=(ci == 0), stop=False,
                                    skip_group_check=True,
                                )
                            for ci, (off, kk) in enumerate(ksizes):
                                nc.tensor.matmul(
                                    ps[:, po:po + ow],
                                    lhsT=sthb[:kk, ci * P:(ci + 1) * P],
                                    rhs=wl_t[ci][:, oo:oo + ow],
                                    start=False, stop=False,
                                    skip_group_check=True,
                                )
                            for ci, (off, kk) in enumerate(ksizes):
                                nc.tensor.matmul(
                                    ps[:, po:po + ow],
                                    lhsT=stl[:kk, ci * P:(ci + 1) * P],
                                    rhs=whb_t[ci][:, oo:oo + ow],
                                    start=False, stop=(ci == n_k - 1),
                                    skip_group_check=True,
                                )
                        else:
                            for ci, (off, kk) in enumerate(ksizes):
                                nc.tensor.matmul(
                                    ps[:, po:po + ow],
                                    lhsT=stb[:kk, ci * P:(ci + 1) * P],
                                    rhs=wh_t[ci][:, oo:oo + ow],
                                    start=(ci == 0),
                                    stop=(grp == "a" and ci == n_k - 1),
                                    skip_group_check=True,
                                )
                            if grp == "c":
                                nc.tensor.matmul(
                                    ps[:, po:po + ow],
                                    lhsT=onesbt[:],
                                    rhs=wcbias[:, oo:oo + ow],
                                    start=False, stop=True,
                                    skip_group_check=True,
                                )
                    if grp == "a":
                        for hh, ps in enumerate(pshs):
                            sb = pac.tile([P, half_w], ytile, tag="ya_sb")
                            nc.scalar.activation(sb[:], ps[:], AF.Copy)
                            nc.sync.dma_start(
                                yad[cs, hh * 864:hh * 864 + half_w], sb[:]
                            )
                    else:
                        for hh, ps in enumerate(pshs):
                            sb = pacy.tile([P, half_w], dt.float32, tag="yc_sb")
                            if split_y:
                                nc.vector.tensor_tensor(
                                    out=sb[:], in0=ps[:],
                                    in1=b1t[:, hh * 864:hh * 864 + half_w],
                                    op=OP.add,
                                )
                            else:
                                nc.scalar.activation(sb[:], ps[:], AF.Copy)
                            nc.sync.dma_start(
                                ycd[cs, hh * 864:hh * 864 + half_w], sb[:]
                            )

                keys_dt = dt.float32
                for t in range(H_TILES):
                    cs = slice(t * P, (t + 1) * P)
                    # ---- coarse gram keys, in (128, 1024) psum quarters
                    keys = pk.tile([P, N], keys_dt, tag="keys")
                    for q in range(4):
                        ps = pgq.tile([P, 1024], dt.float32, tag="gps")
                        for si in range(2):
                            nsl = slice(q * 1024 + si * 512,
                                        q * 1024 + si * 512 + 512)
                            psl = slice(si * 512, si * 512 + 512)
                            for ci, (off, kk) in enumerate(ksizes):
                                nc.tensor.matmul(
                                    ps[:, psl],
                                    lhsT=xtr_t[ci][:kk, cs],
                                    rhs=xtr_t[ci][:kk, nsl],
                                    start=(ci == 0), stop=False,
                                    skip_group_check=True,
                                )
                            nc.tensor.matmul(
                                ps[:, psl],
                                lhsT=onest[:],
                                rhs=biasrow[:, nsl],
                                start=False, stop=True,
                                skip_group_check=True,
                            )
                        nc.scalar.activation(
                            keys[:, q * 1024:(q + 1) * 1024], ps[:], AF.Copy
                        )
                    # ---- coarse top-8
                    top8 = pbs.tile([P, NCAND], keys_dt, tag="top8")
                    nc.vector.max(out=top8[:], in_=keys[:])
                    cidx = pbs.tile([P, NCAND], dt.uint32, tag="cidx")
                    nc.vector.max_index(cidx[:], top8[:], keys[:])
                    cidxf = pbs.tile([P, NCAND], dt.float32, tag="cidxf")
                    nc.vector.tensor_copy(cidxf[:], cidx[:])

                    # ---- exact rerank (dot keys): slot 0 = self via copy
                    cent = pbc.tile([P, xa_w], dt.float32, tag="cent")
                    nc.sync.dma_start(cent[:], xa[cs, :])
                    ekeys = pbs.tile([P, NCAND], dt.float32, tag="ekeys")
                    CH = 4
                    cand = pbc.tile([P, CH, xa_w], dt.float32, tag="cand")
                    nc.scalar.activation(cand[:, 0, :], cent[:], AF.Copy)
                    nc.vector.memset(cent[:, cin:cin + 1], 1.0)
                    for (j0, jn) in ((0, CH), (CH, NCAND - CH)):
                        for j in range(jn):
                            if j0 + j == 0:
                                continue
                            nc.gpsimd.indirect_dma_start(
                                out=cand[:, j, :],
                                out_offset=None,
                                in_=xa[:],
                                in_offset=bass.IndirectOffsetOnAxis(
                                    ap=cidx[:, j0 + j:j0 + j + 1], axis=0
                                ),
                            )
                        nc.gpsimd.tensor_tensor(
                            out=cand[:, :jn, :], in0=cand[:, :jn, :],
                            in1=_mid_bcast(cent[:], jn), op=OP.mult,
                        )
                        nc.vector.tensor_reduce(
                            out=ekeys[:, j0:j0 + jn], in_=cand[:, :jn, :],
                            axis=mybir.AxisListType.X, op=OP.add,
                        )
                    etop = pbs.tile([P, NCAND], dt.float32, tag="etop")
                    nc.vector.max(out=etop[:], in_=ekeys[:])
                    epos = pbs.tile([P, NCAND], dt.uint32, tag="epos")
                    nc.vector.max_index(epos[:], etop[:], ekeys[:])
                    eposf = pbs.tile([P, NCAND], dt.float32, tag="eposf")
                    nc.vector.tensor_copy(eposf[:], epos[:])

                    # ---- exact ranks 1..4 -> original indices
                    KR = K - 1
                    m48 = pbs.tile([P, KR, NCAND], dt.float32, tag="m48")
                    nc.vector.tensor_tensor(
                        out=m48[:], in0=_mid_bcast(io8t[:], KR),
                        in1=_last_bcast(eposf[:, 1:K], NCAND), op=OP.is_equal,
                    )
                    nc.vector.tensor_tensor(
                        out=m48[:], in0=m48[:], in1=_mid_bcast(cidxf[:], KR),
                        op=OP.mult,
                    )
                    idx4f = pbs.tile([P, KR], dt.float32, tag="idx4f")
                    nc.vector.tensor_reduce(
                        out=idx4f[:], in_=m48[:], axis=mybir.AxisListType.X,
                        op=OP.add,
                    )
                    idx4 = pidx.tile([P, KR], dt.uint32, tag="idx4")
                    nc.vector.tensor_copy(idx4[:], idx4f[:])
                    idx4_t.append(idx4)

                    # ---- interleaved y tiles (keeps PE dense and warm)
                    y_tile(2 * t, "a", wa_trio)
                    y_tile(2 * t + 1, "a", wa_trio)
                    if split_y:
                        y_tile(t, "c", wc_trio)

                if not split_y:
                    # yc tail with the wc weights (slots shared via tags)
                    wc_trio = load_w("c")
                    for t in range(H_TILES):
                        y_tile(t, "c", wc_trio)

            tc.strict_bb_all_engine_barrier()

            # ------------- epilogue phase
            with (
                tc.tile_pool(name="pg", bufs=2) as pg,
                tc.tile_pool(name="pe2", bufs=2) as pe2,
            ):
                for t in range(H_TILES):
                    cs = slice(t * P, (t + 1) * P)
                    g5 = pg.tile([P, K, cout], ytile, tag="g5")
                    nc.sync.dma_start(g5[:, 0, :], yad[cs, :])
                    for j in range(K - 1):
                        nc.gpsimd.indirect_dma_start(
                            out=g5[:, j + 1, :],
                            out_offset=None,
                            in_=yad[:],
                            in_offset=bass.IndirectOffsetOnAxis(
                                ap=idx4_t[t][:, j:j + 1], axis=0
                            ),
                        )
                    # max over k: pairwise tree of packed ops (2x/4x DVE modes)
                    mb = pe2.tile([P, cout], ytile, tag="mb")
                    nc.vector.tensor_tensor(out=mb[:], in0=g5[:, 0, :],
                                            in1=g5[:, 1, :], op=OP.max)
                    for j in range(2, K):
                        nc.vector.tensor_tensor(out=mb[:], in0=mb[:],
                                                in1=g5[:, j, :], op=OP.max)
                    yct = pe2.tile([P, cout], dt.float32, tag="yct")
                    nc.sync.dma_start(yct[:], ycd[cs, :])
                    xo = pe2.tile([P, cout], dt.float32, tag="xo")
                    nc.vector.tensor_tensor(out=xo[:], in0=mb[:], in1=yct[:],
                                            op=OP.add)
                    nc.scalar.activation(xo[:], xo[:], AF.Prelu, alpha=SLOPE)
                    nc.sync.dma_start(xout[cs, :], xo[:])

            pidx_cm.__exit__(None, None, None)

    nc.finalize()
    return nc


_CACHE = {}


def _get_programs():
    if "p1" not in _CACHE:
        _CACHE["p1"] = _build_block(C1, O1, XA1, split_y=True)
        _CACHE["p2"] = _build_block(C2, O2, XA2, split_y=False)
    return _CACHE["p1"], _CACHE["p2"]


# ---------------------------------------------------------------- host side

def _fold_bn(W, gamma, beta, mean, var, cin):
    s = gamma.astype(np.float64) / np.sqrt(var.astype(np.float64) + EPS)
    Wp = s[:, None] * W.astype(np.float64)
    Wa = Wp[:, :cin].T
    Wc = (Wp[:, cin:] - Wp[:, :cin]).T
    bp = beta.astype(np.float64) - s * mean.astype(np.float64)
    return (np.ascontiguousarray(Wa, np.float32),
            np.ascontiguousarray(Wc, np.float32),
            bp.astype(np.float32))


def _xtr_aug(xT, sq):
    bias_row = _tf32((-sq / 2).astype(np.float32))[None, :]
    return np.concatenate(
        [_tf32(xT), np.ones((1, N), np.float32), bias_row], axis=0
    )


def _prep_block1(x, Wa, Wc, bp):
    xT = np.ascontiguousarray(x.T)
    sq = np.einsum("nc,nc->n", x.astype(np.float64), x.astype(np.float64))
    xTh = _tf32(xT)
    xa = np.zeros((N, XA1), np.float32)
    xa[:, :C1] = x
    xa[:, C1] = (-sq / 2).astype(np.float32)
    wah = _tf32(Wa)
    wch = _tf32(Wc)
    return dict(
        xTr=_xtr_aug(xT, sq),
        xtl_s=_stripes(_bf16(xT - xTh), len(_chunks(C1))),
        xthb_s=_stripes(_bf16(xTh), len(_chunks(C1))),
        xa=xa,
        wah=wah, wal=_bf16(Wa - wah), wahb=_bf16(wah),
        wch=wch, wcl=_bf16(Wc - wch), wchb=_bf16(wch),
        b1b=np.broadcast_to(bp, (P, O1)).copy(),
        io8=np.broadcast_to(np.arange(NCAND, dtype=np.float32), (P, NCAND)).copy(),
        ones1=np.ones((1, P), np.float32),
    )


def _prep_block2(x1, Wa, Wc, bp):
    xT = np.ascontiguousarray(x1.T)
    sq = np.einsum("nc,nc->n", x1.astype(np.float64), x1.astype(np.float64))
    xa = np.zeros((N, XA2), np.float32)
    xa[:, :C2] = x1
    xa[:, C2] = (-sq / 2).astype(np.float32)
    wch_aug = np.concatenate([Wc, bp[None, :]], axis=0)
    return dict(
        xTr=_xtr_aug(xT, sq),
        x1b_s=_stripes(_bf16(xT), len(_chunks(C2))),
        xa=xa,
        wah=_bf16(Wa),
        wch=_bf16(wch_aug),
        io8=np.broadcast_to(np.arange(NCAND, dtype=np.float32), (P, NCAND)).copy(),
        ones1=np.ones((1, P), np.float32),
        onesb=np.ones((1, P), ml_dtypes.bfloat16),
    )


_LAST_EXEC_NS = {"l1": None, "l2": None}
_X1_DEBUG = {}


def kernel(interm_repr, W1, bn1_gamma, bn1_beta, bn1_mean, bn1_var,
           W2, bn2_gamma, bn2_beta, bn2_mean, bn2_var, _trace=False):
    x = np.asarray(interm_repr, dtype=np.float32)
    p1, p2 = _get_programs()

    W1a, W1c, b1 = _fold_bn(np.asarray(W1), np.asarray(bn1_gamma),
                            np.asarray(bn1_beta), np.asarray(bn1_mean),
                            np.asarray(bn1_var), C1)
    W2a, W2c, b2 = _fold_bn(np.asarray(W2), np.asarray(bn2_gamma),
                            np.asarray(bn2_beta), np.asarray(bn2_mean),
                            np.asarray(bn2_var), C2)

    in_maps = []
    for c in range(8):
        b, h = c // 2, c % 2
        perm = np.r_[h * HALF:(h + 1) * HALF, (1 - h) * HALF:(2 - h) * HALF]
        in_maps.append(_prep_block1(x[b][perm], W1a, W1c, b1))
    r1 = run_bass_kernel_spmd(p1, in_maps, core_ids=list(range(8)), trace=_trace)
    _LAST_EXEC_NS["l1"] = r1.exec_time_ns

    x1 = np.empty((B, N, O1), np.float32)
    for c in range(8):
        b, h = c // 2, c % 2
        x1[b, h * HALF:(h + 1) * HALF] = r1.results[c]["xout"]

    _X1_DEBUG["x1"] = x1
    in_maps = []
    for c in range(8):
        b, h = c // 2, c % 2
        perm = np.r_[h * HALF:(h + 1) * HALF, (1 - h) * HALF:(2 - h) * HALF]
        in_maps.append(_prep_block2(x1[b][perm], W2a, W2c, b2))
    r2 = run_bass_kernel_spmd(p2, in_maps, core_ids=list(range(8)), trace=_trace)
    _LAST_EXEC_NS["l2"] = r2.exec_time_ns

    x2 = np.empty((B, N, O2), np.float32)
    for c in range(8):
        b, h = c // 2, c % 2
        x2[b, h * HALF:(h + 1) * HALF] = r2.results[c]["xout"]
    return x2


if __name__ == "__main__":
    rng = np.random.default_rng(0)
    inp = dict(
        interm_repr=rng.standard_normal((B, N, C1), dtype=np.float32),
        W1=(rng.standard_normal((O1, 2 * C1)) / np.sqrt(2 * C1)).astype(np.float32),
        bn1_gamma=1 + 0.1 * rng.standard_normal(O1).astype(np.float32),
        bn1_beta=0.1 * rng.standard_normal(O1).astype(np.float32),
        bn1_mean=0.1 * rng.standard_normal(O1).astype(np.float32),
        bn1_var=0.5 + rng.random(O1).astype(np.float32),
        W2=(rng.standard_normal((O2, 2 * C2)) / np.sqrt(2 * C2)).astype(np.float32),
        bn2_gamma=1 + 0.1 * rng.standard_normal(O2).astype(np.float32),
        bn2_beta=0.1 * rng.standard_normal(O2).astype(np.float32),
        bn2_mean=0.1 * rng.standard_normal(O2).astype(np.float32),
        bn2_var=0.5 + rng.random(O2).astype(np.float32),
    )
    out = kernel(**inp)
    print("kernel out", out.shape, out.dtype, np.abs(out).mean())



# revision 27
# speedup vs baseline: 1.1793x; 1.1793x over previous
"""Trainium2 Bass kernel for PointCloudTeacher (2x EdgeConv with KNN graph).

Sharding: 8 NeuronCores, B=4 clouds of N=4096 points; core c handles
(batch b = c//2, row-half h = c%2) = 2048 query rows. Per-core inputs are
row-permuted so the core's own half comes first -- one SPMD program serves
all cores. Two launches with a host gather of x1 in between.

Per block:
  - Coarse keys via single-pass f32r (tf32) gram matmul; the -|x_m|^2/2 key
    bias rides an augmented final K-chunk (gq lhsT, device-verified).
  - Coarse top-8 via DVE max8/find_index8 on f16 keys (scaled 1/16).
  - Exact re-rank of candidates 1..6 by f32 dot: per tile, TWO half-row
    dma_gather calls fetch the candidate rows; indices are re-layouted to
    the SWDGE 16-partition-wrapped format (replicated across the 8 Q7
    cores) via two tiny DRAM round-trips. Self key comes free from the
    -sq/2 column. Multiply on GpSimd, reduce on DVE per half.
  - y_a (full cloud) interleaved with the KNN tiles, lagged two tiles;
    y_c is fused into the phase-2 epilogue (PE busy while gathers run).
    The conv bias b' uses the baseline-proven mechanisms (DVE b1t add for
    block1, K=1 ones-matmul for block2) -- NOT the aug-row rider, which
    real HW applies x16.
  - Epilogue: self row of y_a direct-loaded + 4 neighbors via two half-row
    dma_gathers by the exact top-4 indices; max over k via a DVE tree; add
    the y_c PSUM (+bias); LeakyReLU via Prelu.
Precision: block1 y-matmuls use a 3-pass tf32+bf16 hi/lo split with an f32
y_a table and f32 x1 output (block2's KNN graph is hypersensitive to x1
noise); block2 y-matmuls are single-pass bf16 with an f16 y_a table and
f16 output.
"""

import numpy as np
import ml_dtypes

import concourse.bass as bass
import concourse.bacc as bacc
import concourse.mybir as mybir
from concourse.tile import TileContext
from concourse.bass_utils import run_bass_kernel_spmd

dt = mybir.dt
AF = mybir.ActivationFunctionType
OP = mybir.AluOpType

P = 128
N = 4096
HALF = 2048
B = 4
C1, O1 = 512, 864
C2, O2 = 864, 1728
K = 5
NCAND = 7            # coarse candidates kept (incl. self); 6 gathered
EPS = 1e-5
SLOPE = 0.2
XA1 = 576            # rerank row width block1 (pad to 256B multiple)
XA2 = 896            # rerank row width block2
YP1 = 896            # yad padded width block1 (f32; 3584B rows)
YP2 = 1792           # yad padded width block2 (f16; 3584B rows)
N_TILES = N // P     # 32
H_TILES = HALF // P  # 16
KEYS_SCALE = 1.0 / 16.0


# ---------------------------------------------------------------- host utils

def _tf32(a):
    a = np.ascontiguousarray(a, dtype=np.float32)
    u = a.view(np.uint32).astype(np.uint64)
    u = ((u + 0x1000 + ((u >> 13) & 1)) & 0xFFFFE000).astype(np.uint32)
    return u.view(np.float32)


def _bf16(a):
    return np.ascontiguousarray(a, dtype=np.float32).astype(ml_dtypes.bfloat16)


def _stripes(xT, n_k):
    """(rows, N) -> (N_TILES, 128, n_k*128) tile-major lhsT stripes."""
    rows, n = xT.shape
    out = np.zeros((N_TILES, P, n_k * P), xT.dtype)
    for ci in range(n_k):
        kk = min(P, rows - ci * P)
        blk = xT[ci * P:ci * P + kk, :].reshape(kk, N_TILES, P)
        out[:, :kk, ci * P:(ci + 1) * P] = blk.transpose(1, 0, 2)
    return out


def _mid_bcast(ap, rep):
    pat = [list(ap.ap[0]), [0, rep], list(ap.ap[1])]
    return bass.AP(ap.tensor, ap.offset, pat)


def _last_bcast(ap, rep):
    pat = [list(ap.ap[0]), list(ap.ap[1]), [0, rep]]
    return bass.AP(ap.tensor, ap.offset, pat)


def _chunks(c):
    out = []
    o = 0
    while o < c:
        kk = min(P, c - o)
        out.append((o, kk))
        o += kk
    return out


# ---------------------------------------------------------------- program

def _build_block(cin, cout, xa_w, ypad_w, split_y):
    nc = bacc.Bacc("TRN2", target_bir_lowering=False)

    caug = cin + 2
    ks = _chunks(caug)
    n_k = len(ks)
    lofft, lkk = ks[-1]
    ytile = dt.float32 if split_y else dt.float16
    xout_dt = dt.float32 if split_y else dt.float16
    NG = NCAND - 1
    SR = NG * 8
    KR = K - 1
    SE = KR * 8
    if xa_w == XA1:
        xh1, xh2 = 320, 256
    else:
        xh1, xh2 = 448, 448
    yh = ypad_w // 2

    # ---- inputs
    xTr = nc.dram_tensor("xTr", [caug, N], dt.float32r, kind="ExternalInput")
    gq_w = P if lkk == 2 else HALF
    gqd = nc.dram_tensor("gqd", [lkk, gq_w], dt.float32r, kind="ExternalInput")
    xa = nc.dram_tensor("xa", [N, xa_w], dt.float32, kind="ExternalInput")
    io8 = nc.dram_tensor("io8", [P, 8], dt.float32, kind="ExternalInput")
    if split_y:
        xtl_s = nc.dram_tensor("xtl_s", [N_TILES, P, n_k * P], dt.bfloat16,
                               kind="ExternalInput")
        xthb_s = nc.dram_tensor("xthb_s", [N_TILES, P, n_k * P], dt.bfloat16,
                                kind="ExternalInput")
        wah = nc.dram_tensor("wah", [caug, cout], dt.float32r, kind="ExternalInput")
        wal = nc.dram_tensor("wal", [caug, cout], dt.bfloat16, kind="ExternalInput")
        wahb = nc.dram_tensor("wahb", [caug, cout], dt.bfloat16, kind="ExternalInput")
        wch = nc.dram_tensor("wch", [caug, cout], dt.float32r, kind="ExternalInput")
        wcl = nc.dram_tensor("wcl", [caug, cout], dt.bfloat16, kind="ExternalInput")
        wchb = nc.dram_tensor("wchb", [caug, cout], dt.bfloat16, kind="ExternalInput")
        b1b = nc.dram_tensor("b1b", [P, cout], dt.float32, kind="ExternalInput")
    else:
        x1b_s = nc.dram_tensor("x1b_s", [N_TILES, P, n_k * P], dt.bfloat16,
                               kind="ExternalInput")
        wah = nc.dram_tensor("wah", [caug, cout], dt.bfloat16, kind="ExternalInput")
        wch = nc.dram_tensor("wch", [caug, cout], dt.bfloat16, kind="ExternalInput")
        wcb = nc.dram_tensor("wcb", [1, cout], dt.bfloat16, kind="ExternalInput")
        onesb = nc.dram_tensor("onesb", [1, P], dt.bfloat16, kind="ExternalInput")

    yad = nc.dram_tensor("yad", [N, ypad_w], ytile)
    xout = nc.dram_tensor("xout", [HALF, cout], xout_dt, kind="ExternalOutput")

    if cout == O2:
        osub = [(0, 512), (512, 352), (864, 512), (1376, 352)]
    else:
        osub = [(0, 512), (512, 352)]
    half_w = min(cout, 864)
    n_half = (cout + 863) // 864

    with TileContext(nc) as tc:
        with (
            tc.tile_pool(name="pers", bufs=1) as pers,
            tc.tile_pool(name="pdram", bufs=2, space="DRAM") as pdram,
        ):
            io8t = pers.tile([P, 8], dt.float32, tag="io8")
            nc.sync.dma_start(io8t[:], io8[:])
            if not split_y:
                onesbt = pers.tile([1, P], dt.bfloat16, tag="onesbt")
                nc.sync.dma_start(onesbt[:], onesb[:])

            piwe_cm = tc.tile_pool(name="piwe", bufs=H_TILES)
            piwe = piwe_cm.__enter__()
            iwe_t = []

            px_cm = None
            if split_y:
                px = pers
            else:
                px_cm = tc.tile_pool(name="px", bufs=1)
                px = px_cm.__enter__()
            xtr_t = []
            for ci, (off, kk) in enumerate(ks):
                t = px.tile([kk, N], dt.float32r, tag=f"xtr{ci}")
                nc.sync.dma_start(t[:], xTr[off:off + kk, :])
                xtr_t.append(t)
            gqt = px.tile([lkk, gq_w], dt.float32r, tag="gqt")
            nc.sync.dma_start(gqt[:], gqd[:])

            def load_w(grp, pool):
                wsrc = wah if grp == "a" else wch
                wdt = dt.float32r if split_y else dt.bfloat16
                wh_t, wl_t, whb_t = [], [], []
                for ci, (off, kk) in enumerate(ks):
                    t = pool.tile([kk, cout], wdt, tag=f"w{grp}h{ci}")
                    nc.sync.dma_start(t[:], wsrc[off:off + kk, :])
                    wh_t.append(t)
                    if split_y:
                        wsl = wal if grp == "a" else wcl
                        wsb = wahb if grp == "a" else wchb
                        t = pool.tile([kk, cout], dt.bfloat16, tag=f"w{grp}l{ci}")
                        nc.sync.dma_start(t[:], wsl[off:off + kk, :])
                        wl_t.append(t)
                        t = pool.tile([kk, cout], dt.bfloat16, tag=f"w{grp}hb{ci}")
                        nc.sync.dma_start(t[:], wsb[off:off + kk, :])
                        whb_t.append(t)
                return wh_t, wl_t, whb_t

            def y_matmuls(tile, trio, stripe_pool, psum_pool, grp):
                """Matmuls for one (128, cout) y tile; returns psum tiles.
                grp 'c' on block2 adds the K=1 ones x wcb bias matmul."""
                cs = slice(tile * P, (tile + 1) * P)
                wh_t, wl_t, whb_t = trio
                if split_y:
                    stl = stripe_pool.tile([P, n_k * P], dt.bfloat16, tag="stl")
                    nc.sync.dma_start(stl[:], xtl_s[tile])
                    sthb = stripe_pool.tile([P, n_k * P], dt.bfloat16, tag="sthb")
                    nc.sync.dma_start(sthb[:], xthb_s[tile])
                else:
                    stb = stripe_pool.tile([P, n_k * P], dt.bfloat16, tag="stb")
                    nc.sync.dma_start(stb[:], x1b_s[tile])
                pshs = [psum_pool.tile([P, half_w], dt.float32, tag="yps",
                                       name=f"yps{hh}")
                        for hh in range(n_half)]
                add_bias = (not split_y) and grp == "c"
                # y-matmuls use only the REAL x rows: the last chunk is
                # sliced to kk-2 (aug rows excluded). Baseline never had a
                # tiny-K y-matmul; K=2 chunks read garbage on real HW.
                yks = [(ci, kk - 2 if ci == n_k - 1 else kk)
                       for ci, (off, kk) in enumerate(ks)]
                yks = [(ci, kk) for ci, kk in yks if kk > 0]
                last_ci = yks[-1][0]
                for (oo, ow) in osub:
                    hh, po = (0, oo) if oo < 864 else (1, oo - 864)
                    ps = pshs[hh]
                    if split_y:
                        for ci, kk in yks:
                            nc.tensor.matmul(
                                ps[:, po:po + ow],
                                lhsT=xtr_t[ci][:kk, cs],
                                rhs=wh_t[ci][:kk, oo:oo + ow],
                                start=(ci == 0), stop=False,
                                skip_group_check=True,
                            )
                        for ci, kk in yks:
                            nc.tensor.matmul(
                                ps[:, po:po + ow],
                                lhsT=sthb[:kk, ci * P:(ci + 1) * P],
                                rhs=wl_t[ci][:kk, oo:oo + ow],
                                start=False, stop=False,
                                skip_group_check=True,
                            )
                        for ci, kk in yks:
                            nc.tensor.matmul(
                                ps[:, po:po + ow],
                                lhsT=stl[:kk, ci * P:(ci + 1) * P],
                                rhs=whb_t[ci][:kk, oo:oo + ow],
                                start=False, stop=(ci == last_ci),
                                skip_group_check=True,
                            )
                    else:
                        for ci, kk in yks:
                            nc.tensor.matmul(
                                ps[:, po:po + ow],
                                lhsT=stb[:kk, ci * P:(ci + 1) * P],
                                rhs=wh_t[ci][:kk, oo:oo + ow],
                                start=(ci == 0),
                                stop=(ci == last_ci) and not add_bias,
                                skip_group_check=True,
                            )
                        if add_bias:
                            nc.tensor.matmul(
                                ps[:, po:po + ow],
                                lhsT=onesbt[:],
                                rhs=wcbt[:, oo:oo + ow],
                                start=False, stop=True,
                                skip_group_check=True,
                            )
                return pshs

            # ------------- phase 1: KNN + y_a
            with (
                tc.tile_pool(name="pw", bufs=1) as pw,
                tc.tile_pool(name="pstr", bufs=2) as pstr,
                tc.tile_pool(name="pk", bufs=2) as pk,
                tc.tile_pool(name="pbs", bufs=2) as pbs,
                tc.tile_pool(name="pc1", bufs=1) as pc1,
                tc.tile_pool(name="pc2", bufs=1) as pc2,
                tc.tile_pool(name="pcent", bufs=2) as pcent,
                tc.tile_pool(name="pac", bufs=3) as pac,
                tc.tile_pool(name="pgq", bufs=2, space="PSUM") as pgq,
                tc.tile_pool(name="pyp", bufs=2, space="PSUM") as pyp,
            ):
                wa_trio = load_w("a", pw)

                def y_tile_a(tile):
                    pshs = y_matmuls(tile, wa_trio, pstr, pyp, "a")
                    cs = slice(tile * P, (tile + 1) * P)
                    for hh, ps in enumerate(pshs):
                        sb = pac.tile([P, half_w], ytile, tag="ya_sb")
                        nc.scalar.activation(sb[:], ps[:], AF.Copy)
                        nc.sync.dma_start(
                            yad[cs, hh * 864:hh * 864 + half_w], sb[:]
                        )

                for t in range(H_TILES):
                    cs = slice(t * P, (t + 1) * P)
                    keys = pk.tile([P, N], dt.float16, tag="keys")
                    for q in range(4):
                        ps = pgq.tile([P, 1024], dt.float32, tag="gps")
                        for si in range(2):
                            nsl = slice(q * 1024 + si * 512,
                                        q * 1024 + si * 512 + 512)
                            psl = slice(si * 512, si * 512 + 512)
                            for ci, (off, kk) in enumerate(ks):
                                if ci == n_k - 1:
                                    gcs = (slice(0, P) if gq_w == P else cs)
                                    lhs = gqt[:lkk, gcs]
                                else:
                                    lhs = xtr_t[ci][:kk, cs]
                                nc.tensor.matmul(
                                    ps[:, psl],
                                    lhsT=lhs,
                                    rhs=xtr_t[ci][:kk, nsl],
                                    start=(ci == 0), stop=(ci == n_k - 1),
                                    skip_group_check=True,
                                )
                        nc.scalar.activation(
                            keys[:, q * 1024:(q + 1) * 1024], ps[:], AF.Copy,
                            scale=KEYS_SCALE,
                        )
                    top8 = pbs.tile([P, 8], dt.float16, tag="top8")
                    nc.vector.max(out=top8[:], in_=keys[:])
                    cidx = pbs.tile([P, 8], dt.uint32, tag="cidx")
                    nc.vector.max_index(cidx[:], top8[:], keys[:])
                    cidxf = pbs.tile([P, 8], dt.float32, tag="cidxf")
                    nc.vector.tensor_copy(cidxf[:], cidx[:])
                    cidx16 = pbs.tile([P, NG], dt.int16, tag="cidx16")
                    nc.vector.tensor_copy(cidx16[:], cidx[:, 1:1 + NG])

                    sr1 = pdram.tile([P, NG], dt.int16, tag="sr1")
                    nc.sync.dma_start(sr1[:], cidx16[:])
                    srw = pbs.tile([16, NG, 8], dt.int16, tag="srw")
                    b1 = sr1[:]
                    nc.sync.dma_start(
                        srw[:, :, :],
                        bass.AP(b1.tensor, b1.offset,
                                [[NG, 16], [1, NG], [16 * NG, 8]]),
                    )
                    sr2 = pdram.tile([16, SR], dt.int16, tag="sr2")
                    nc.sync.dma_start(sr2[:], srw[:, :, :])
                    iwr = pbs.tile([P, NG, 8], dt.int16, tag="iwr")
                    b2 = sr2[:]
                    nc.sync.dma_start(
                        iwr[:, :, :],
                        bass.AP(b2.tensor, b2.offset,
                                [[0, 8], [SR, 16], [1, SR]]),
                    )

                    cent = pcent.tile([P, xa_w], dt.float32, tag="cent")
                    nc.sync.dma_start(cent[:], xa[cs, :])
                    ekeys = pbs.tile([P, 8], dt.float32, tag="ekeys")
                    nc.vector.tensor_scalar_mul(
                        ekeys[:, 0:1], cent[:, cin:cin + 1], -1.0
                    )
                    nc.vector.memset(ekeys[:, NCAND:8], -3.0e38)
                    nc.vector.memset(cent[:, cin:cin + 1], 1.0)
                    ekh = pbs.tile([P, NG], dt.float32, tag="ekh")
                    for hi, (pool, co, cw) in enumerate(
                        ((pc1, 0, xh1), (pc2, xh1, xh2))
                    ):
                        cnd = pool.tile([P, NG, cw], dt.float32, tag=f"cand{hi}")
                        nc.gpsimd.dma_gather(
                            out_ap=cnd[:, :, :],
                            in_ap=xa[:, co:co + cw],
                            idxs_ap=iwr[:, :, :],
                            num_idxs=P * NG,
                            num_idxs_reg=P * NG,
                            elem_size=cw,
                            elem_step=xa_w,
                        )
                        nc.gpsimd.tensor_tensor(
                            out=cnd[:, :, :], in0=cnd[:, :, :],
                            in1=_mid_bcast(cent[:, co:co + cw], NG),
                            op=OP.mult,
                        )
                        if hi == 0:
                            nc.vector.tensor_reduce(
                                out=ekh[:], in_=cnd[:, :, :],
                                axis=mybir.AxisListType.X, op=OP.add,
                            )
                        else:
                            nc.vector.tensor_reduce(
                                out=ekeys[:, 1:1 + NG], in_=cnd[:, :, :],
                                axis=mybir.AxisListType.X, op=OP.add,
                            )
                            nc.vector.tensor_tensor(
                                out=ekeys[:, 1:1 + NG],
                                in0=ekeys[:, 1:1 + NG], in1=ekh[:],
                                op=OP.add,
                            )
                    etop = pbs.tile([P, 8], dt.float32, tag="etop")
                    nc.vector.max(out=etop[:], in_=ekeys[:])
                    epos = pbs.tile([P, 8], dt.uint32, tag="epos")
                    nc.vector.max_index(epos[:], etop[:], ekeys[:])
                    eposf = pbs.tile([P, 8], dt.float32, tag="eposf")
                    nc.vector.tensor_copy(eposf[:], epos[:])

                    m48 = pbs.tile([P, KR, 8], dt.float32, tag="m48")
                    nc.vector.tensor_tensor(
                        out=m48[:], in0=_mid_bcast(io8t[:], KR),
                        in1=_last_bcast(eposf[:, 1:K], 8), op=OP.is_equal,
                    )
                    nc.vector.tensor_tensor(
                        out=m48[:], in0=m48[:], in1=_mid_bcast(cidxf[:], KR),
                        op=OP.mult,
                    )
                    idx4f = pbs.tile([P, KR], dt.float32, tag="idx4f")
                    nc.vector.tensor_reduce(
                        out=idx4f[:], in_=m48[:], axis=mybir.AxisListType.X,
                        op=OP.add,
                    )
                    idx4u = pbs.tile([P, KR], dt.uint32, tag="idx4u")
                    nc.vector.tensor_copy(idx4u[:], idx4f[:])
                    idx416 = pbs.tile([P, KR], dt.int16, tag="idx416")
                    nc.vector.tensor_copy(idx416[:], idx4u[:])

                    se1 = pdram.tile([P, KR], dt.int16, tag="se1")
                    nc.sync.dma_start(se1[:], idx416[:])
                    sew = pbs.tile([16, KR, 8], dt.int16, tag="sew")
                    b3 = se1[:]
                    nc.sync.dma_start(
                        sew[:, :, :],
                        bass.AP(b3.tensor, b3.offset,
                                [[KR, 16], [1, KR], [16 * KR, 8]]),
                    )
                    se2 = pdram.tile([16, SE], dt.int16, tag="se2")
                    nc.sync.dma_start(se2[:], sew[:, :, :])
                    iwe = piwe.tile([P, KR, 8], dt.int16, tag="iwe")
                    b4 = se2[:]
                    nc.sync.dma_start(
                        iwe[:, :, :],
                        bass.AP(b4.tensor, b4.offset,
                                [[0, 8], [SE, 16], [1, SE]]),
                    )
                    iwe_t.append(iwe)

                    if t >= 2:
                        y_tile_a(2 * (t - 2))
                        y_tile_a(2 * (t - 2) + 1)
                for t in range(H_TILES - 2, H_TILES):
                    y_tile_a(2 * t)
                    y_tile_a(2 * t + 1)

            if px_cm is not None:
                px_cm.__exit__(None, None, None)

            # ------------- phase 2: y_c + epilogue
            with (
                tc.tile_pool(name="pw2", bufs=1) as pw2,
                tc.tile_pool(name="pstr2", bufs=2) as pstr2,
                tc.tile_pool(name="pg", bufs=2) as pg,
                tc.tile_pool(name="pgh1", bufs=2) as pgh1,
                tc.tile_pool(name="pgh2", bufs=2) as pgh2,
                tc.tile_pool(name="pe2", bufs=2) as pe2,
                tc.tile_pool(name="pyc", bufs=2, space="PSUM") as pyc,
            ):
                wc_trio = load_w("c", pw2)
                if split_y:
                    b1t = pw2.tile([P, cout], dt.float32, tag="b1t")
                    nc.sync.dma_start(b1t[:], b1b[:])
                    wcbt = None
                else:
                    wcbt = pw2.tile([1, cout], dt.bfloat16, tag="wcbt")
                    nc.sync.dma_start(wcbt[:], wcb[:])
                pre_n = 2 if n_half == 1 else 1
                ycps_pre = {t: y_matmuls(t, wc_trio, pstr2, pyc, "c")
                            for t in range(pre_n)}

                tc.strict_bb_all_engine_barrier()

                for t in range(H_TILES):
                    cs = slice(t * P, (t + 1) * P)
                    ycps = (ycps_pre.get(t)
                            or y_matmuls(t, wc_trio, pstr2, pyc, "c"))
                    gs = pg.tile([P, cout], ytile, tag="gself")
                    nc.sync.dma_start(gs[:], yad[cs, 0:cout])
                    gh = []
                    for hi, pool in ((0, pgh1), (1, pgh2)):
                        g5 = pool.tile([P, KR, yh], ytile, tag=f"g5{hi}")
                        nc.gpsimd.dma_gather(
                            out_ap=g5[:, :, :],
                            in_ap=yad[:, hi * yh:(hi + 1) * yh],
                            idxs_ap=iwe_t[t][:, :, :],
                            num_idxs=P * KR,
                            num_idxs_reg=P * KR,
                            elem_size=yh,
                            elem_step=ypad_w,
                        )
                        gh.append(g5)
                    mb = pe2.tile([P, cout], ytile, tag="mb")
                    for hi, g5 in enumerate(gh):
                        co = hi * yh
                        cw = min(yh, cout - co)
                        if cw <= 0:
                            continue
                        ms = slice(co, co + cw)
                        nc.vector.tensor_tensor(out=mb[:, ms], in0=gs[:, ms],
                                                in1=g5[:, 0, 0:cw], op=OP.max)
                        for j in range(1, KR):
                            nc.vector.tensor_tensor(out=mb[:, ms],
                                                    in0=mb[:, ms],
                                                    in1=g5[:, j, 0:cw],
                                                    op=OP.max)
                    xo = pe2.tile([P, cout], dt.float32, tag="xo")
                    for hh, ps in enumerate(ycps):
                        hs = slice(hh * 864, hh * 864 + half_w)
                        if split_y:
                            # baseline-proven: psum + b1t -> sbuf, then + mb
                            nc.vector.tensor_tensor(out=xo[:, hs], in0=ps[:],
                                                    in1=b1t[:, hs], op=OP.add)
                            nc.vector.tensor_tensor(out=xo[:, hs],
                                                    in0=xo[:, hs],
                                                    in1=mb[:, hs], op=OP.add)
                        else:
                            nc.vector.tensor_tensor(out=xo[:, hs],
                                                    in0=mb[:, hs],
                                                    in1=ps[:], op=OP.add)
                    xo2 = pe2.tile([P, cout], xout_dt, tag="xo2")
                    nc.scalar.activation(xo2[:], xo[:], AF.Prelu, alpha=SLOPE)
                    nc.sync.dma_start(xout[cs, :], xo2[:])

            piwe_cm.__exit__(None, None, None)

    nc.finalize()
    return nc


_CACHE = {}


def _get_programs():
    if "p1" not in _CACHE:
        _CACHE["p1"] = _build_block(C1, O1, XA1, YP1, split_y=True)
        _CACHE["p2"] = _build_block(C2, O2, XA2, YP2, split_y=False)
    return _CACHE["p1"], _CACHE["p2"]


# ---------------------------------------------------------------- host side

def _fold_bn(W, gamma, beta, mean, var, cin):
    s = gamma.astype(np.float64) / np.sqrt(var.astype(np.float64) + EPS)
    Wp = s[:, None] * W.astype(np.float64)
    Wa = Wp[:, :cin].T
    Wc = (Wp[:, cin:] - Wp[:, :cin]).T
    bp = beta.astype(np.float64) - s * mean.astype(np.float64)
    return (np.ascontiguousarray(Wa, np.float32),
            np.ascontiguousarray(Wc, np.float32),
            bp.astype(np.float32))


def _prep(x, Wa, Wc, bp, cin, cout, xa_w, split_y):
    caug = cin + 2
    ks = _chunks(caug)
    n_k = len(ks)
    lofft, lkk = ks[-1]

    xT = np.ascontiguousarray(x.T)
    sq = np.einsum("nc,nc->n", x.astype(np.float64), x.astype(np.float64))
    biasrow = (-sq / 2).astype(np.float32)
    aug = np.concatenate(
        [xT, biasrow[None, :], np.ones((1, N), np.float32)], axis=0
    )
    augh = _tf32(aug)

    if lkk == 2:
        gq = np.concatenate([np.ones((1, P), np.float32),
                             np.zeros((1, P), np.float32)], axis=0)
    else:
        gq = np.concatenate(
            [augh[lofft:cin, :HALF], np.ones((1, HALF), np.float32),
             np.zeros((1, HALF), np.float32)], axis=0)
    assert gq.shape[0] == lkk

    # aug weight rows are ZERO (the aug-row bias rider is broken on HW:
    # it applies x16); bias is delivered via b1b / wcb instead.
    zrow = np.zeros((2, cout), np.float64)
    wa_aug = np.concatenate([Wa.astype(np.float64), zrow], axis=0)
    wc_aug = np.concatenate([Wc.astype(np.float64), zrow], axis=0)

    xa = np.zeros((N, xa_w), np.float32)
    xa[:, :cin] = x
    xa[:, cin] = biasrow

    m = dict(
        xTr=augh,
        gqd=gq,
        xa=xa,
        io8=np.broadcast_to(np.arange(8, dtype=np.float32), (P, 8)).copy(),
    )
    if split_y:
        wahv = _tf32(wa_aug.astype(np.float32))
        wchv = _tf32(wc_aug.astype(np.float32))
        m.update(
            xtl_s=_stripes(_bf16(aug - augh), n_k),
            xthb_s=_stripes(_bf16(augh), n_k),
            wah=wahv, wal=_bf16(wa_aug - wahv.astype(np.float64)),
            wahb=_bf16(wahv),
            wch=wchv, wcl=_bf16(wc_aug - wchv.astype(np.float64)),
            wchb=_bf16(wchv),
            b1b=np.broadcast_to(bp, (P, cout)).copy(),
        )
    else:
        m.update(
            x1b_s=_stripes(_bf16(aug), n_k),
            wah=_bf16(wa_aug.astype(np.float32)),
            wch=_bf16(wc_aug.astype(np.float32)),
            wcb=_bf16(bp[None, :]),
            onesb=np.ones((1, P), ml_dtypes.bfloat16),
        )
    return m


_LAST_EXEC_NS = {"l1": None, "l2": None}


def kernel(interm_repr, W1, bn1_gamma, bn1_beta, bn1_mean, bn1_var,
           W2, bn2_gamma, bn2_beta, bn2_mean, bn2_var, _trace=False):
    x = np.asarray(interm_repr, dtype=np.float32)
    p1, p2 = _get_programs()

    W1a, W1c, b1 = _fold_bn(np.asarray(W1), np.asarray(bn1_gamma),
                            np.asarray(bn1_beta), np.asarray(bn1_mean),
                            np.asarray(bn1_var), C1)
    W2a, W2c, b2 = _fold_bn(np.asarray(W2), np.asarray(bn2_gamma),
                            np.asarray(bn2_beta), np.asarray(bn2_mean),
                            np.asarray(bn2_var), C2)

    in_maps = []
    for c in range(8):
        b, h = c // 2, c % 2
        perm = np.r_[h * HALF:(h + 1) * HALF, (1 - h) * HALF:(2 - h) * HALF]
        in_maps.append(_prep(x[b][perm], W1a, W1c, b1, C1, O1, XA1, True))
    r1 = run_bass_kernel_spmd(p1, in_maps, core_ids=list(range(8)), trace=_trace)
    _LAST_EXEC_NS["l1"] = r1.exec_time_ns

    x1 = np.empty((B, N, O1), np.float32)
    for c in range(8):
        b, h = c // 2, c % 2
        x1[b, h * HALF:(h + 1) * HALF] = r1.results[c]["xout"]

    in_maps = []
    for c in range(8):
        b, h = c // 2, c % 2
        perm = np.r_[h * HALF:(h + 1) * HALF, (1 - h) * HALF:(2 - h) * HALF]
        in_maps.append(_prep(x1[b][perm], W2a, W2c, b2, C2, O2, XA2, False))
    r2 = run_bass_kernel_spmd(p2, in_maps, core_ids=list(range(8)), trace=_trace)
    _LAST_EXEC_NS["l2"] = r2.exec_time_ns

    x2 = np.empty((B, N, O2), np.float32)
    for c in range(8):
        b, h = c // 2, c % 2
        x2[b, h * HALF:(h + 1) * HALF] = \
            r2.results[c]["xout"].astype(np.float32)
    return x2


if __name__ == "__main__":
    rng = np.random.default_rng(0)
    inp = dict(
        interm_repr=rng.standard_normal((B, N, C1), dtype=np.float32),
        W1=(rng.standard_normal((O1, 2 * C1)) / np.sqrt(2 * C1)).astype(np.float32),
        bn1_gamma=1 + 0.1 * rng.standard_normal(O1).astype(np.float32),
        bn1_beta=0.1 * rng.standard_normal(O1).astype(np.float32),
        bn1_mean=0.1 * rng.standard_normal(O1).astype(np.float32),
        bn1_var=0.5 + rng.random(O1).astype(np.float32),
        W2=(rng.standard_normal((O2, 2 * C2)) / np.sqrt(2 * C2)).astype(np.float32),
        bn2_gamma=1 + 0.1 * rng.standard_normal(O2).astype(np.float32),
        bn2_beta=0.1 * rng.standard_normal(O2).astype(np.float32),
        bn2_mean=0.1 * rng.standard_normal(O2).astype(np.float32),
        bn2_var=0.5 + rng.random(O2).astype(np.float32),
    )
    out = kernel(**inp)
    print("kernel out", out.shape, out.dtype, np.abs(out).mean())
